# revision 21
# baseline (speedup 1.0000x reference)
"""Trainium2 Bass kernel for a Neural CDE (GunnarODE).

Full-input contract: kernel(**inputs) takes the complete (unsharded) inputs
and returns the complete (L, B, Y) output. Internally the batch dim (B=256)
is sharded across 8 NeuronCores (32 per core); the small MLP weights are
replicated. The sequential 127-step x 2-substep Euler scan runs on-device.

v5 design notes (changes vs v4):
- Output is decoded incrementally: per step, 2 small matmuls (z @ dec_W)
  + one DVE add land rows in a (step,b)-partition staging tile; every 4
  steps one dense contiguous 8KB DMA ships them out. This removes the
  ~180us serial strided-DMA tail v4 paid after the scan.
- State is bf16-only (zTr [128, 2*BC]); the fp32 master + its 2 extra DVE
  adds per substep are gone. One DVE add per substep closes the loop.
- relu runs on DVE (tensor_scalar max), with the b1 bias folded into the
  GEMM1 PSUM accumulation via a K=2 matmul (b1 rows x half-selector).
- b2 bias lands via 5 full-width K=4 selector matmuls (one per c-group)
  instead of 17 col-tiled ones.
- Decode matmuls + next-substep bias matmuls fill the PE tail gaps so the
  HAM clock gate stays at 2.4 GHz (v4 oscillated 4/8 <-> 8/8 every ~20us).
"""

import sys

for _p in ("/opt/trn_rl_repo", "/root/.axon_site/_ro/trn_rl_repo"):
    if _p not in sys.path:
        sys.path.append(_p)

import numpy as np
import ml_dtypes

import concourse.bass as bass
import concourse.bacc as bacc
import concourse.mybir as mybir
from concourse.tile import TileContext
from concourse.bass_utils import run_bass_kernel_spmd

# Problem dims (hardcoded per contract)
L, B, H, C, Y = 128, 256, 256, 17, 16
N_SUB = 2
NCORES = 8
BC = B // NCORES           # batch per core = 32
NSTEPS = L - 1             # 127 scan steps
NSUBSTEPS = NSTEPS * N_SUB # 254
NG = 5                     # c-groups: [4,4,4,4,1] (g4 = c16 singleton)
GF = 256                   # free width of one c-group block in G
F32 = mybir.dt.float32
BF16 = mybir.dt.bfloat16

AluOp = mybir.AluOpType
Act = mybir.ActivationFunctionType

# chunk -> groups; group -> (chunk, offset within chunk tile)
CHUNKS = {"A": [4], "B": [0, 1], "C": [2, 3]}
GPLACE = {4: ("A", 0), 0: ("B", 0), 1: ("B", GF), 2: ("C", 0), 3: ("C", GF)}


def build_bass(n_substeps=NSUBSTEPS, warmup=50, dump=False):
    """Build the per-core Bass program (same program for all cores)."""
    nc = bacc.Bacc("TRN2", target_bir_lowering=False, debug=False)

    w1s = nc.dram_tensor("w1s", [128, 512], BF16, kind="ExternalInput")
    w2s = nc.dram_tensor("w2s", [128, 2 * C * 256], BF16, kind="ExternalInput")
    b2s = nc.dram_tensor("b2s", [128, C * 256], BF16, kind="ExternalInput")
    ones = nc.dram_tensor("ones", [128, BC], BF16, kind="ExternalInput")
    b1k = nc.dram_tensor("b1k", [2, 128], BF16, kind="ExternalInput")
    b1c = nc.dram_tensor("b1c", [128, 2], F32, kind="ExternalInput")
    ones2 = nc.dram_tensor("ones2", [2, 2 * BC], BF16, kind="ExternalInput")
    zt0 = nc.dram_tensor("zt0", [128, 2 * BC], BF16, kind="ExternalInput")
    dcol = nc.dram_tensor("dcol", [128, NSUBSTEPS * NG], F32, kind="ExternalInput")
    maskd = nc.dram_tensor("maskd", [128, NG * BC], BF16, kind="ExternalInput")
    decw = nc.dram_tensor("decw", [128, 2 * Y], BF16, kind="ExternalInput")
    decb = nc.dram_tensor("decb", [BC, Y], F32, kind="ExternalInput")
    out = nc.dram_tensor("out", [L, BC, Y], F32, kind="ExternalOutput")
    outv = out[:].rearrange("(g s) b y -> g (s b) y", s=4)  # 32 output chunks

    with TileContext(nc) as tc:
        with (
            tc.tile_pool(name="const", bufs=1) as constp,
            tc.tile_pool(name="work", bufs=2) as work,
            tc.tile_pool(name="state", bufs=1) as statep,
            tc.tile_pool(name="ob", bufs=2) as obp,
            tc.tile_pool(name="psG", bufs=1, space="PSUM") as psG,
            tc.tile_pool(name="psH", bufs=1, space="PSUM") as psH,
            tc.tile_pool(name="psZ", bufs=1, space="PSUM") as psZ,
            tc.tile_pool(name="psD", bufs=1, space="PSUM") as psD,
            tc.tile_pool(name="psF", bufs=1, space="PSUM") as psF,
        ):
            w1 = constp.tile([128, 512], BF16)
            nc.sync.dma_start(w1[:], w1s[:])
            b1t = constp.tile([2, 128], BF16)
            nc.sync.dma_start(b1t[:], b1k[:])
            b1t2 = constp.tile([128, 2], F32)
            nc.sync.dma_start(b1t2[:], b1c[:])
            onet = constp.tile([2, 2 * BC], BF16)
            nc.sync.dma_start(onet[:], ones2[:])
            onest = constp.tile([128, BC], BF16)
            nc.sync.dma_start(onest[:], ones[:])
            w2 = constp.tile([128, 2 * C * 256], BF16)
            nc.sync.dma_start(w2[:], w2s[:])
            b2t = constp.tile([128, C * 256], BF16)
            nc.sync.dma_start(b2t[:], b2s[:])
            maskt = constp.tile([128, NG * BC], BF16)
            nc.sync.dma_start(maskt[:], maskd[:])
            dcolt = constp.tile([128, NSUBSTEPS * NG], F32)
            nc.sync.dma_start(dcolt[:], dcol[:])
            decwt = constp.tile([128, 2 * Y], BF16)
            nc.sync.dma_start(decwt[:], decw[:])
            decbt = constp.tile([BC, Y], F32)
            nc.sync.dma_start(decbt[:], decb[:])

            # bf16 state, double-buffered by substep parity. zt[0] always
            # holds "state before an even substep" = the per-step states.
            zt = [statep.tile([128, 2 * BC], BF16, name=f"zt{i}") for i in range(2)]
            nc.sync.dma_start(zt[0][:], zt0[:])

            # ---- HAM warmup: dummy matmuls on a memset tile (no DMA
            # dependency, so they start immediately and overlap the input
            # DMA phase) keep the PE busy until the scan starts warm.
            wusrc = constp.tile([128, GF], BF16, name="wusrc")
            nc.vector.memset(wusrc[:], 1.0)
            wuP = psG.tile([128, GF], F32, tag="gPA")
            for i in range(warmup):
                nc.tensor.matmul(
                    wuP[0:BC, 0:GF],
                    wusrc[:, 0:BC],
                    wusrc[:],
                    start=True,
                    stop=True,
                    skip_group_check=True,
                )

            def emit_bias(g, gP, off, start=True):
                # gP[(jj,b), h2] = b2[4g+jj, h2] via eye-matmul (K=128 so
                # the HAM activity monitor sees full-array work).
                # Only the FIRST bias matmul into a PSUM tile uses
                # start=True: start clears has_written for the whole bank,
                # so a second start=True would wipe the first region's bias
                # (a start=False write onto cleared has_written overwrites,
                # which is what we want for later regions).
                for jj in range(4 if g < 4 else 1):
                    c = 4 * g + jj
                    nc.tensor.matmul(
                        gP[32 * jj : 32 * jj + 32, off : off + GF],
                        onest[:],
                        b2t[:, c * 256 : (c + 1) * 256],
                        start=start,
                        stop=False,
                        tile_position=(0, 32 * jj),
                        skip_group_check=True,
                    )

            def emit_mains(g, gP, off, hdnt):
                n_jj = 4 if g < 4 else 1
                for kh in range(2):
                    for jj in range(n_jj):
                        c = 4 * g + jj
                        nc.tensor.matmul(
                            gP[32 * jj : 32 * jj + 32, off : off + GF],
                            hdnt[:, kh * BC : (kh + 1) * BC],
                            w2[:, kh * C * 256 + c * 256 : kh * C * 256 + c * 256 + 256],
                            start=False,
                            stop=(kh == 1),
                            tile_position=(0, 32 * jj),
                            skip_group_check=True,
                        )

            def emit_contract(g, hh, gchunk, zdT, start, stop, dkt):
                # transpose-mode matmul: out = gS_sliceT @ dk_g lands the
                # delta already transposed (h on partitions) in PSUM
                _, off = GPLACE[g]
                if g < 4:
                    lhs = gchunk[:, off + hh * 128 : off + hh * 128 + 128]
                    rhs = dkt[:, g * BC : (g + 1) * BC]
                else:
                    lhs = gchunk[0:BC, off + hh * 128 : off + hh * 128 + 128]
                    rhs = dkt[0:BC, g * BC : (g + 1) * BC]
                nc.tensor.matmul(
                    zdT[:, hh * BC : (hh + 1) * BC],
                    lhs,
                    rhs,
                    is_transpose=True,
                    start=start,
                    stop=stop,
                    skip_group_check=True,
                )

            def emit_decode(step, ob):
                # decP[b, y] = sum_h z[h, b] * dec_W[h, y]  (z = zt[0])
                decP = psD.tile([BC, Y], F32, tag="decP")
                for hh in range(2):
                    nc.tensor.matmul(
                        decP[:, :],
                        zt[0][:, hh * BC : (hh + 1) * BC],
                        decwt[:, hh * Y : (hh + 1) * Y],
                        start=(hh == 0),
                        stop=(hh == 1),
                        skip_group_check=True,
                    )
                sl = step % 4
                nc.vector.tensor_tensor(
                    ob[32 * sl : 32 * sl + 32, :], decP[:, :], decbt[:], AluOp.add
                )

            # staging tile for 4 output rows; chunk g covers steps 4g..4g+3
            ob = obp.tile([128, Y], F32, tag="ob")
            emit_decode(0, ob)  # row 0 = z0 decode

            def emit_fil(fil):
                nc.tensor.matmul(
                    fil[0:BC, :],
                    w1[:, 0:BC],
                    w2[:, 0:GF],
                    start=True,
                    stop=True,
                    skip_group_check=True,
                )

            # v7 rotated schedule: each body k runs GEMM1 -> relu -> mains
            # + tanh -> contracts(all) -> state add; the NEXT substep's
            # biases are emitted right after the tanh that frees each gP
            # tile, so they fill the PE wait-gaps. Fillers sit just before
            # each PE wait point to keep the HAM clock gate at 2.4 GHz.
            gPA = psG.tile([128, GF], F32, tag="gPA")
            gPB = psG.tile([128, 2 * GF], F32, tag="gPB")
            gPC = psG.tile([128, 2 * GF], F32, tag="gPC")
            emit_bias(4, gPA, 0)
            emit_bias(0, gPB, 0)
            emit_bias(1, gPB, GF, start=False)
            emit_bias(2, gPC, 0)
            emit_bias(3, gPC, GF, start=False)

            for k in range(n_substeps):
                zc, zn = zt[k % 2], zt[(k + 1) % 2]

                # 1. hdnP b1 init (PE prime at k=0; DVE broadcast after)
                hdnP = psH.tile([128, 2 * BC], F32, tag="hdnP", name="hdnP")
                if k == 0:
                    nc.tensor.matmul(
                        hdnP[:, :],
                        b1t[0:2, :],
                        onet[0:2, :],
                        start=True,
                        stop=False,
                        skip_group_check=True,
                    )
                else:
                    nc.vector.tensor_copy(
                        hdnP[:].rearrange("p (m b) -> p m b", b=BC),
                        b1t2[:].unsqueeze(2).broadcast_to([128, 2, BC]),
                    )

                # 2. dk on GPSIMD (keeps the DVE queue clear for the adds)
                dk = work.tile([128, NG * BC], BF16, tag="dk")
                dk3 = dk[:].rearrange("p (g b) -> p g b", b=BC)
                mask3 = maskt[:].rearrange("p (g b) -> p g b", b=BC)
                dc3 = (
                    dcolt[:, k * NG : (k + 1) * NG]
                    .unsqueeze(2)
                    .broadcast_to([128, NG, BC])
                )
                nc.gpsimd.tensor_tensor(dk3, mask3, dc3, AluOp.mult)

                # 3. GEMM1 strips accumulate onto the b1-initialized PSUM
                for mh in range(2):
                    for kh in range(2):
                        for jj in range(4):
                            nc.tensor.matmul(
                                hdnP[32 * jj : 32 * jj + 32, mh * BC : (mh + 1) * BC],
                                w1[:, (kh * 2 + mh) * 128 + 32 * jj : (kh * 2 + mh) * 128 + 32 * jj + 32],
                                zc[:, kh * BC : (kh + 1) * BC],
                                start=False,
                                stop=(kh == 1),
                                tile_position=(0, 32 * jj),
                                skip_group_check=True,
                            )

                # 4. relu on DVE: PE<->DVE sem hops measure ~5x faster
                # than PE<->ACT hops, so this beats a one-call ACT relu.
                hdnt = work.tile([128, 2 * BC], BF16, tag="hdn", name="hdn")
                nc.vector.tensor_scalar(hdnt[:], hdnP[:], 0.0, None, AluOp.max)

                # 5. mains + eager per-chunk tanh
                gSA = work.tile([128, GF], BF16, tag="gSA")
                gSB = work.tile([128, 2 * GF], BF16, tag="gSB")
                gSC = work.tile([128, 2 * GF], BF16, tag="gSC")
                emit_mains(4, gPA, 0, hdnt)
                nc.scalar.activation(gSA[0:BC, :], gPA[0:BC, :], Act.Tanh)
                emit_mains(0, gPB, 0, hdnt)
                emit_mains(1, gPB, GF, hdnt)
                nc.scalar.activation(gSB[:], gPB[:], Act.Tanh)
                emit_mains(2, gPC, 0, hdnt)
                emit_mains(3, gPC, GF, hdnt)
                nc.scalar.activation(gSC[:], gPC[:], Act.Tanh)

                # 6. incremental decode: state after substep k-1 (odd) is
                # step k//2; PE executes these during the tanh stream.
                if k >= 2 and k % 2 == 0:
                    step = k // 2
                    if step % 4 == 0:
                        ob = obp.tile([128, Y], F32, tag="ob")
                    emit_decode(step, ob)
                    if step % 4 == 3:
                        nc.sync.dma_start(outv[step // 4], ob[:])

                # 7. contracts A, B after a filler (PE waits tanh-B here)
                fil = psF.tile([128, GF], F32, tag="fil", name="fil")
                emit_fil(fil)
                zdT = psZ.tile([128, 2 * BC], BF16, tag="zdT", name="zdT")
                for hh in range(2):
                    # start=True only on the very first write to the tile
                    emit_contract(4, hh, gSA, zdT, hh == 0, False, dk)
                    emit_contract(0, hh, gSB, zdT, False, False, dk)
                    emit_contract(1, hh, gSB, zdT, False, False, dk)

                # 8. next substep's bias A/B (gP tiles freed by tanh A/B)
                gPA = psG.tile([128, GF], F32, tag="gPA")
                gPB = psG.tile([128, 2 * GF], F32, tag="gPB")
                emit_bias(4, gPA, 0)
                emit_bias(0, gPB, 0)
                emit_bias(1, gPB, GF, start=False)

                # 9. contracts C after a filler (PE waits tanh-C here)
                fil = psF.tile([128, GF], F32, tag="fil", name="fil")
                emit_fil(fil)
                for hh in range(2):
                    emit_contract(2, hh, gSC, zdT, False, False, dk)
                    emit_contract(3, hh, gSC, zdT, False, True, dk)

                # 10. next substep's bias C + state update on DVE
                gPC_n = psG.tile([128, 2 * GF], F32, tag="gPC")
                emit_bias(2, gPC_n, 0)
                emit_bias(3, gPC_n, GF, start=False)
                fil = psF.tile([128, GF], F32, tag="fil", name="fil")
                emit_fil(fil)
                nc.vector.tensor_add(zn[:], zc[:], zdT[:])
                gPC = gPC_n

                if dump and k == n_substeps - 1:
                    dump.update(hdn=hdnt, dk=dk, gSA=gSA, gSB=gSB, gSC=gSC,
                                hdnP=hdnP, zdT=zdT, zf=zn)

            # epilogue: decode step 127 + last output chunk
            if dump:
                for nm, tl, shp, dt_ in [
                    ("dbg_z", dump["zf"], [128, 64], BF16),
                    ("dbg_hdn", dump["hdn"], [128, 64], BF16),
                    ("dbg_dk", dump["dk"], [128, NG * BC], BF16),
                    ("dbg_gSA", dump["gSA"], [128, GF], BF16),
                    ("dbg_gSB", dump["gSB"], [128, 2 * GF], BF16),
                    ("dbg_gSC", dump["gSC"], [128, 2 * GF], BF16),
                    ("dbg_zd", dump["zdT"], [128, 64], BF16),
                    ("dbg_hP", dump["hdnP"], [128, 64], F32),
                ]:
                    drt = nc.dram_tensor(nm, shp, dt_, kind="ExternalOutput")
                    if nm in ("dbg_zd", "dbg_hP"):
                        tmp = statep.tile(shp, dt_, name=f"tmp_{nm}")
                        nc.vector.tensor_copy(tmp[:], tl[:])
                        nc.sync.dma_start(drt[:], tmp[:])
                    elif nm == "dbg_gSA":
                        nc.sync.dma_start(drt[0:BC], tl[0:BC, :])
                    else:
                        nc.sync.dma_start(drt[:], tl[:])
            if n_substeps == NSUBSTEPS:
                step = n_substeps // 2
                emit_decode(step, ob)
                nc.sync.dma_start(outv[step // 4], ob[:])

    nc.compile()
    return nc


def host_prep(ts, us, enc_b, f_W1, f_b1, f_W2, f_b2, dec_W, dec_b, n_substeps=NSUBSTEPS):
    """Host-side packing of weights + spline-derivative scalars."""
    ts = np.asarray(ts, np.float64)
    us = np.asarray(us, np.float64)
    t = ts[:, 0, 0]
    dt = t[1:] - t[:-1]                                  # (L-1,)
    x = np.concatenate([ts, us], axis=-1).transpose(1, 0, 2)  # (B, L, C)
    h = dt[None, :, None]
    slope = (x[:, 1:] - x[:, :-1]) / h
    m = np.concatenate([slope[:, :1], slope], axis=1)
    mi, mn = m[:, :-1], m[:, 1:]
    xi, xn = x[:, :-1], x[:, 1:]
    c2 = 3.0 * (xn - xi) / h**2 - (2.0 * mi + mn) / h
    c3 = 2.0 * (xi - xn) / h**3 + (mi + mn) / h**2
    dX0 = mi                                             # u = 0
    dX1 = mi + c2 * h + 0.75 * c3 * h * h                # u = h/2
    scale = h / N_SUB                                    # (1, L-1, 1)
    dxs = np.stack([dX0 * scale, dX1 * scale], axis=2)   # (B, L-1, 2, C)
    dxs = dxs.transpose(1, 2, 0, 3).reshape(NSUBSTEPS, B, C).astype(np.float32)

    f_W1 = np.asarray(f_W1, np.float32)
    f_W2 = np.asarray(f_W2, np.float32)
    f_b1 = np.asarray(f_b1, np.float32)
    f_b2 = np.asarray(f_b2, np.float32)
    enc_b = np.asarray(enc_b, np.float32)
    dec_W = np.asarray(dec_W, np.float32)
    dec_b = np.asarray(dec_b, np.float32)

    # W1 packed: w1s[p, (kh*2+mh)*128 + m] = W1[kh*128+p, mh*128+m]
    w1s = np.zeros((128, 512), np.float32)
    for kh in range(2):
        for mh in range(2):
            w1s[:, (kh * 2 + mh) * 128 : (kh * 2 + mh + 1) * 128] = f_W1[
                kh * 128 : (kh + 1) * 128, mh * 128 : (mh + 1) * 128
            ]

    # W2 c-major: w2s[p, kh*C*256 + c*256 + h2] = W2[kh*128+p, h2*C + c]
    w2r = f_W2.reshape(H, H, C)                          # [h_in, h_out, c]
    w2cm = w2r.transpose(0, 2, 1).reshape(H, C * H)      # [h_in, c, h_out]
    w2s = np.concatenate([w2cm[:128], w2cm[128:]], axis=1)  # (128, 2*C*256)

    # b2 c-major, broadcast to all partitions: b2s[p, c*256 + h2] = b2[h2*C + c]
    b2r = f_b2.reshape(H, C)                             # [h_out, c]
    b2cm = b2r.T.reshape(1, C * H)                       # [c, h_out]
    b2sp = np.broadcast_to(b2cm, (128, C * H)).copy()

    # b1k rows = b1 halves; ones2 selects the mh free-block
    b1kp = np.stack([f_b1[:128], f_b1[128:]], axis=0)    # (2, 128)
    ones2 = np.zeros((2, 2 * BC), np.float32)
    ones2[0, :BC] = 1.0
    ones2[1, BC:] = 1.0

    z0 = enc_b                                            # zeros @ enc_W + enc_b
    zt0 = np.zeros((128, 2 * BC), np.float32)
    for hh in range(2):
        zt0[:, hh * BC : (hh + 1) * BC] = z0[hh * 128 : (hh + 1) * 128][:, None]

    # mask[32*jj + bb, g*BC + bb'] = (bb == bb') for groups with c = 4g+jj < C
    maskd = np.zeros((128, NG * BC), np.float32)
    bb = np.arange(BC)
    for g in range(NG):
        for jj in range(4 if g < 4 else 1):
            maskd[32 * jj + bb, g * BC + bb] = 1.0

    # dcol[32*jj + bb, k*NG + g] = dxs[k, core*BC + bb, 4g+jj]
    dcol_cores = []
    for core in range(NCORES):
        d = np.zeros((128, NSUBSTEPS * NG), np.float32)
        for g in range(NG):
            for jj in range(4 if g < 4 else 1):
                c = 4 * g + jj
                d[32 * jj + bb[:, None], np.arange(n_substeps)[None, :] * NG + g] = dxs[
                    :n_substeps, core * BC + bb, c
                ].T
        dcol_cores.append(d)

    decw = np.concatenate([dec_W[:128], dec_W[128:]], axis=1).astype(np.float32)  # (128, 2Y)
    decb = np.broadcast_to(dec_b[None, :], (BC, Y)).copy().astype(np.float32)

    common = {
        "w1s": w1s.astype(ml_dtypes.bfloat16),
        "w2s": w2s.astype(ml_dtypes.bfloat16),
        "b2s": b2sp.astype(ml_dtypes.bfloat16),
        "ones": np.eye(128, BC, dtype=np.float32).astype(ml_dtypes.bfloat16),
        "b1k": b1kp.astype(ml_dtypes.bfloat16),
        "b1c": np.stack([f_b1[:128], f_b1[128:]], axis=1).astype(np.float32),
        "ones2": ones2.astype(ml_dtypes.bfloat16),
        "zt0": zt0.astype(ml_dtypes.bfloat16),
        "maskd": maskd.astype(ml_dtypes.bfloat16),
        "decw": decw.astype(ml_dtypes.bfloat16),
        "decb": decb,
    }
    in_maps = []
    for core in range(NCORES):
        m_ = dict(common)
        m_["dcol"] = dcol_cores[core]
        in_maps.append(m_)
    return in_maps


_CACHE = {}


def _get_nc(n_substeps=NSUBSTEPS):
    key = n_substeps
    if key not in _CACHE:
        _CACHE[key] = build_bass(n_substeps)
    return _CACHE[key]


def run(inputs, n_substeps=NSUBSTEPS, trace=False, **kw):
    in_maps = host_prep(
        inputs["ts"], inputs["us"], inputs["enc_b"], inputs["f_W1"],
        inputs["f_b1"], inputs["f_W2"], inputs["f_b2"], inputs["dec_W"],
        inputs["dec_b"], n_substeps=n_substeps,
    )
    nc = _get_nc(n_substeps)
    res = run_bass_kernel_spmd(nc, in_maps, core_ids=list(range(NCORES)), trace=trace, **kw)
    outs = [np.asarray(res.results[i]["out"]) for i in range(NCORES)]
    full = np.concatenate(outs, axis=1)  # (L, B, Y)
    return full, res


def kernel(**inputs) -> np.ndarray:
    full, _ = run(inputs)
    return full.astype(np.float32)
